# revision 2
# baseline (speedup 1.0000x reference)
"""Trainium2 Bass kernel for nn_CrossAttention (b=8, n=2048, dim=768, inner=512).

Strategy
--------
Data-parallel over batch: 8 batches -> 8 NeuronCores, no collectives.

Per core (one batch), with all activations pre-transposed on host so every
matmul has its contraction dim on SBUF partitions:

  qpT[d,n] = (8*Wq.T).T-style proj:  matmul(lhsT=wq8T[c,d], rhs=qT[c,n])   fp32
  kpT[d,m] = matmul(lhsT=wkT[c,d],  rhs=kT[c,n])                            fp32
  vpT[d,m] = matmul(lhsT=wvT[c,d],  rhs=vT[c,n])                            bf16
  vpW[m,c] = matmul(lhsT=vpT[d,m],  rhs=wpT[d,c])   (associativity:
             out = P @ (vp @ Wp.T), so the output projection folds into
             the value matrix once instead of once per row-tile)          bf16
  S[n,m]   = matmul(lhsT=qpT slice, rhs=kpT)  (x8 folded into wq8T)        fp32
  P        = exp(S - rowmax)  (ACT, accum_out gives rowsum)                bf16
  PT       = PE-transpose of P tiles                                       bf16
  out[n,c] = matmul(lhsT=PT, rhs=vpW) * (1/rowsum)                         bf16

fp32 is required on the q/k/S path: logits have sigma~60 (the module
multiplies logits by 8) and reduced-precision matmuls (fp32r: 1.5e-4 rel,
bf16: 2.3e-3 rel, both HW-measured) inject absolute logit noise that
perturbs the post-softmax output too much.  The value path is smooth, so
bf16 is fine there.
"""

import numpy as np
import ml_dtypes

from concourse import bacc
import concourse.bass as bass
import concourse.mybir as mybir
import concourse.tile as tile
from concourse.bass_utils import run_bass_kernel_spmd
from concourse.masks import make_identity

P = 128          # partitions
N = 2048         # sequence length (n == m)
C = 768          # model dim
D = 512          # inner dim
KC = C // P      # 6 contraction tiles over c
DT = D // P      # 4 tiles over d
NT = N // P      # 16 row tiles
NCH = 4          # 512-wide chunks for projections
CW = N // NCH    # 512

f32 = mybir.dt.float32
bf16 = mybir.dt.bfloat16
AX = mybir.AxisListType.X
EXP = mybir.ActivationFunctionType.Exp

_NC_CACHE = {}


def _build():
    nc = bacc.Bacc("TRN2", target_bir_lowering=False, debug=False, num_devices=8)

    qTh_d = nc.dram_tensor("qTh", [C, N], bf16, kind="ExternalInput")
    qTl_d = nc.dram_tensor("qTl", [C, N], bf16, kind="ExternalInput")
    kTh_d = nc.dram_tensor("kTh", [C, N], bf16, kind="ExternalInput")
    kTl_d = nc.dram_tensor("kTl", [C, N], bf16, kind="ExternalInput")
    vT_d = nc.dram_tensor("vT", [C, N], bf16, kind="ExternalInput")
    wqh_d = nc.dram_tensor("wqTh", [C, D], bf16, kind="ExternalInput")  # 8*Wq.T hi
    wql_d = nc.dram_tensor("wqTl", [C, D], bf16, kind="ExternalInput")  # 8*Wq.T lo
    wkh_d = nc.dram_tensor("wkTh", [C, D], bf16, kind="ExternalInput")
    wkl_d = nc.dram_tensor("wkTl", [C, D], bf16, kind="ExternalInput")
    wv_d = nc.dram_tensor("wvT", [C, D], bf16, kind="ExternalInput")  # Wv.T
    wp_d = nc.dram_tensor("wpT", [D, C], bf16, kind="ExternalInput")  # Wp.T
    out_d = nc.dram_tensor("out", [N, C], f32, kind="ExternalOutput")

    with tile.TileContext(nc) as tc:
        with (
            tc.tile_pool(name="wpool", bufs=1) as wpool,
            tc.tile_pool(name="big", bufs=1) as big,
            tc.tile_pool(name="xs", bufs=4) as xs,
            tc.tile_pool(name="pp", bufs=2) as ppool,
            tc.tile_pool(name="pts", bufs=2) as ptsp,
            tc.tile_pool(name="ob", bufs=2) as obp,
            tc.tile_pool(name="st", bufs=4) as stp,
        ):
            # ---- weights ----
            wqh = wpool.tile([P, KC, D], bf16)
            nc.sync.dma_start(wqh[:], wqh_d.rearrange("(b p) d -> p b d", p=P))
            wql = wpool.tile([P, KC, D], bf16)
            nc.sync.dma_start(wql[:], wql_d.rearrange("(b p) d -> p b d", p=P))
            wkh = wpool.tile([P, KC, D], bf16)
            nc.sync.dma_start(wkh[:], wkh_d.rearrange("(b p) d -> p b d", p=P))
            wkl = wpool.tile([P, KC, D], bf16)
            nc.sync.dma_start(wkl[:], wkl_d.rearrange("(b p) d -> p b d", p=P))
            wv = wpool.tile([P, KC, D], bf16)
            nc.sync.dma_start(wv[:], wv_d.rearrange("(b p) d -> p b d", p=P))
            wp = wpool.tile([P, DT, C], bf16)
            nc.sync.dma_start(wp[:], wp_d.rearrange("(t p) c -> p t c", p=P))
            ident = wpool.tile([P, P], bf16)
            make_identity(nc, ident[:])

            # ---- big SBUF residents ----
            qpTh = big.tile([P, DT, N], bf16)  # [d_sub, dt, n] hi
            qpTl = big.tile([P, DT, N], bf16)  # lo
            kpTh = big.tile([P, DT, N], bf16)
            kpTl = big.tile([P, DT, N], bf16)
            vpT = big.tile([P, DT, N], bf16)   # [d_sub, dt, m]
            vpW = big.tile([P, NT, C], bf16)   # [m_sub, mt, c]

            # ---- phase A: projections (k, v, vpW, then q) ----
            def proj_pair_chunk(hi_d, lo_d, wh, wl, dsth, dstl, ch, psum_pool):
                xh = xs.tile([P, KC, CW], bf16, tag="xchunk")
                nc.sync.dma_start(
                    xh[:], hi_d[:, ch * CW:(ch + 1) * CW].rearrange(
                        "(b p) n -> p b n", p=P))
                xl = xs.tile([P, KC, CW], bf16, tag="xchunk")
                nc.sync.dma_start(
                    xl[:], lo_d[:, ch * CW:(ch + 1) * CW].rearrange(
                        "(b p) n -> p b n", p=P))
                for dt_ in range(DT):
                    ps = psum_pool.tile([P, CW], f32, tag="mm")
                    n_mm = KC * 3
                    idx = 0
                    for cb in range(KC):
                        for wt, xt in ((wh, xh), (wl, xh), (wh, xl)):
                            nc.tensor.matmul(
                                ps[:],
                                wt[:, cb, dt_ * P:(dt_ + 1) * P],
                                xt[:, cb, :],
                                start=(idx == 0),
                                stop=(idx == n_mm - 1),
                            )
                            idx += 1
                    hs = dsth[:, dt_, ch * CW:(ch + 1) * CW]
                    nc.vector.tensor_copy(hs, ps[:])
                    nc.vector.tensor_sub(
                        dstl[:, dt_, ch * CW:(ch + 1) * CW], ps[:], hs)

            def proj_chunk(src_d, w, dst, dst_dt, ch, psum_pool):
                x = xs.tile([P, KC, CW], src_d.dtype, tag="xchunk")
                nc.sync.dma_start(
                    x[:], src_d[:, ch * CW:(ch + 1) * CW].rearrange(
                        "(b p) n -> p b n", p=P)
                )
                for dt_ in range(DT):
                    ps = psum_pool.tile([P, CW], f32, tag="mm")
                    for cb in range(KC):
                        nc.tensor.matmul(
                            ps[:],
                            w[:, cb, dt_ * P:(dt_ + 1) * P],
                            x[:, cb, :],
                            start=(cb == 0),
                            stop=(cb == KC - 1),
                        )
                    nc.vector.tensor_copy(
                        dst[:, dt_, ch * CW:(ch + 1) * CW], ps[:]
                    )

            with tc.tile_pool(name="psA", bufs=2, space="PSUM") as psA:
                for ch in range(NCH):
                    proj_pair_chunk(kTh_d, kTl_d, wkh, wkl, kpTh, kpTl, ch, psA)
                for ch in range(NCH):
                    proj_chunk(vT_d, wv, vpT, bf16, ch, psA)
                    # vpW tiles for the m-range this chunk covers
                    for mt in range(ch * 4, ch * 4 + 4):
                        pa = psA.tile([P, D], f32, tag="vwa")
                        pb = psA.tile([P, C - D], f32, tag="vwb")
                        for dt_ in range(DT):
                            st_ = (dt_ == 0)
                            sp_ = (dt_ == DT - 1)
                            nc.tensor.matmul(
                                pa[:], vpT[:, dt_, mt * P:(mt + 1) * P],
                                wp[:, dt_, 0:D], start=st_, stop=sp_)
                            nc.tensor.matmul(
                                pb[:], vpT[:, dt_, mt * P:(mt + 1) * P],
                                wp[:, dt_, D:C], start=st_, stop=sp_)
                        nc.vector.tensor_copy(vpW[:, mt, 0:D], pa[:])
                        nc.vector.tensor_copy(vpW[:, mt, D:C], pb[:])
                for ch in range(NCH):
                    proj_pair_chunk(qTh_d, qTl_d, wqh, wql, qpTh, qpTl, ch, psA)

            # ---- phase B: attention per row tile ----
            with (
                tc.tile_pool(name="psS", bufs=1, space="PSUM") as psS,
                tc.tile_pool(name="psScr", bufs=2, space="PSUM") as psScr,
                tc.tile_pool(name="psO", bufs=1, space="PSUM") as psO,
            ):
                for i in range(NT):
                    S = psS.tile([P, N], f32, tag="S")
                    for mch in range(NCH):
                        n_mm = DT * 3
                        idx = 0
                        for dt_ in range(DT):
                            for lt, rt in (
                                (qpTh, kpTh), (qpTh, kpTl), (qpTl, kpTh)
                            ):
                                nc.tensor.matmul(
                                    S[:, mch * CW:(mch + 1) * CW],
                                    lt[:, dt_, i * P:(i + 1) * P],
                                    rt[:, dt_, mch * CW:(mch + 1) * CW],
                                    start=(idx == 0),
                                    stop=(idx == n_mm - 1),
                                )
                                idx += 1
                    negmax = stp.tile([P, 1], f32, tag="negmax")
                    nc.vector.reduce_max(negmax[:], S[:], axis=AX, negate=True)
                    Pt = ppool.tile([P, N], bf16, tag="P")
                    sumexp = stp.tile([P, 1], f32, tag="sum")
                    nc.scalar.activation(
                        Pt[:], S[:], EXP, bias=negmax[:], scale=1.0,
                        accum_out=sumexp[:],
                    )
                    # transpose P in two 8-tile batches
                    PTs = ptsp.tile([P, N], bf16, tag="PTs")
                    for h in range(2):
                        tp = psScr.tile([P, N // 2], bf16, tag="scr")
                        for u in range(8):
                            mt = h * 8 + u
                            nc.tensor.transpose(
                                tp[:, u * P:(u + 1) * P],
                                Pt[:, mt * P:(mt + 1) * P],
                                ident[:],
                            )
                        nc.vector.tensor_copy(
                            PTs[:, h * (N // 2):(h + 1) * (N // 2)], tp[:]
                        )
                    oa = psO.tile([P, D], f32, tag="oa")
                    ob = psO.tile([P, C - D], f32, tag="ob")
                    for mt in range(NT):
                        st_ = (mt == 0)
                        sp_ = (mt == NT - 1)
                        nc.tensor.matmul(
                            oa[:], PTs[:, mt * P:(mt + 1) * P],
                            vpW[:, mt, 0:D], start=st_, stop=sp_)
                        nc.tensor.matmul(
                            ob[:], PTs[:, mt * P:(mt + 1) * P],
                            vpW[:, mt, D:C], start=st_, stop=sp_)
                    inv = stp.tile([P, 1], f32, tag="inv")
                    nc.vector.reciprocal(inv[:], sumexp[:])
                    osb = obp.tile([P, C], f32, tag="osb")
                    nc.scalar.mul(osb[:, 0:D], oa[:], inv[:])
                    nc.scalar.mul(osb[:, D:C], ob[:], inv[:])
                    nc.sync.dma_start(out_d[i * P:(i + 1) * P, :], osb[:])

    nc.compile()
    return nc


def _get_nc():
    if "nc" not in _NC_CACHE:
        _NC_CACHE["nc"] = _build()
    return _NC_CACHE["nc"]


def _split_bf16(x):
    hi = x.astype(ml_dtypes.bfloat16)
    lo = (x - hi.astype(np.float32)).astype(ml_dtypes.bfloat16)
    return hi, lo


def _make_in_maps(q, k, v, Wq, Wk, Wv, Wp):
    q = np.asarray(q, dtype=np.float32)
    k = np.asarray(k, dtype=np.float32)
    v = np.asarray(v, dtype=np.float32)
    wq8 = np.ascontiguousarray(np.asarray(Wq, dtype=np.float32).T) * np.float32(8.0)
    wk = np.ascontiguousarray(np.asarray(Wk, dtype=np.float32).T)
    wqh, wql = _split_bf16(wq8)
    wkh, wkl = _split_bf16(wk)
    wv = np.asarray(Wv, dtype=np.float32).T.astype(ml_dtypes.bfloat16)
    wp = np.asarray(Wp, dtype=np.float32).T.astype(ml_dtypes.bfloat16)
    in_maps = []
    for b in range(8):
        qh, ql = _split_bf16(np.ascontiguousarray(q[b].T))
        kh, kl = _split_bf16(np.ascontiguousarray(k[b].T))
        in_maps.append({
            "qTh": qh, "qTl": ql,
            "kTh": kh, "kTl": kl,
            "vT": v[b].T.astype(ml_dtypes.bfloat16),
            "wqTh": wqh, "wqTl": wql,
            "wkTh": wkh, "wkTl": wkl,
            "wvT": wv,
            "wpT": wp,
        })
    return in_maps


def kernel(q, k, v, Wq, Wk, Wv, Wp):
    nc = _get_nc()
    in_maps = _make_in_maps(q, k, v, Wq, Wk, Wv, Wp)
    res = run_bass_kernel_spmd(nc, in_maps, list(range(8)))
    return np.stack([res.results[i]["out"] for i in range(8)], axis=0)


def kernel_traced(q, k, v, Wq, Wk, Wv, Wp, **trace_kwargs):
    """Like kernel() but profiles the NEFF; returns (out, BassKernelResults)."""
    nc = _get_nc()
    in_maps = _make_in_maps(q, k, v, Wq, Wk, Wv, Wp)
    res = run_bass_kernel_spmd(
        nc, in_maps, list(range(8)), trace=True, **trace_kwargs
    )
    out = np.stack([res.results[i]["out"] for i in range(8)], axis=0)
    return out, res
